# revision 52
# baseline (speedup 1.0000x reference)
"""Trainium2 Bass kernel for ConstrainedAttentionModel (sparse_attention).

Full-input contract: kernel(x=[8,2048] int, C=[4,4] f32) -> [8,2048] f32.
Data parallel across 8 NeuronCores: one batch row per core.

Math (per row, T=2048, k=4, V=2048):
  scores[t] = sum_{i,j} C[i,j] * [x[t-j] == x[T-1-i]]   (t-j >= 0)
  scores[T-1] = -1e9; attn = softmax(scores)
  out[v] = sum_t attn[t] * [x[t] == v]

Device strategy (t = 16p + f layout on 128 partitions):
  - all operands arrive pre-converted to fp16 (token ids < 2048 are
    exact in fp16), so the DVE pipeline needs zero prep casts:
      sync DMA:   per-partition [19-token window | lo=x&63 | hi=x-lo]
                  (shifted lag-j window copies are overlapping SBUF
                  views, stride -1 on j)
      scalar DMA: 64B q+C row (single partition)
      gpsimd:     iota rows (lo row + f-replicated hi table)
  - q+C broadcast to 128 partitions via one K=1 PE matmul + one fp16
    PSUM->SBUF copy in an idle DVE slot
  - equality/score chain in fp16 (packed 2x DVE mode), fused reduce,
    single exp -> E fp16
  - vocab one-hot factorized as v = 64*hi + lo:
      A[hi,f] = E[f]*[x>>6 == hi]  transposed layout -> both the
        compare (vs the replicated hi table) and the E-multiply hit the
        2x DVE mode; per-f lhsT slices are strided (LDWEIGHTS is fine)
      B[f,lo] = [x&63 == lo]  contiguous (matmul rhs must stream packed)
    out[hi,lo] = 16 PSUM-accumulated fp16 matmuls at full PE rate
  - the t=T-1 (softmax-masked) position is excluded by contracting only
    127 partitions in the last matmul
  - the kernel returns the UNNORMALIZED histogram; softmax
    normalization happens on host: out = y / y.sum() (Z == sum(y))
"""
import os
import numpy as np
import concourse.bass as bass
import concourse.bacc as bacc
import concourse.tile as tile
from concourse import mybir

P = 128
T = 2048
F = T // P  # 16
K = 4
FH = F // 2  # 8
NHI = 32
NLO = 64
XW = F + K - 1  # 19

fp32 = mybir.dt.float32
fp16 = mybir.dt.float16
Alu = mybir.AluOpType
Act = mybir.ActivationFunctionType

N_WARM1 = int(os.environ.get("KERNEL_N_WARM1", "4"))
N_WARM2 = int(os.environ.get("KERNEL_N_WARM2", "4"))

B = 8


def _build_nc():
    nc = bacc.Bacc()
    NW = XW + 2 * F  # 51: [window 19 | lo 16 | hi 16] per partition
    xw = nc.dram_tensor("xw", [P * NW], fp16, kind="ExternalInput")
    qc = nc.dram_tensor("qc", [P * 32], fp16, kind="ExternalInput")
    y = nc.dram_tensor("y", [T], fp16, kind="ExternalOutput")

    with tile.TileContext(nc) as tc:
        with (
            tc.tile_pool(name="sb", bufs=1) as sb,
            tc.tile_pool(name="ps", bufs=1, space="PSUM") as ps,
        ):
            XB = sb.tile([P, NW], fp16)  # per-partition [win|lo|hi]
            QC = sb.tile([P, 32], fp16)  # host-replicated q+C per partition

            nc.sync.dma_start(
                out=XB[:],
                in_=bass.AP(tensor=xw[:].tensor, offset=0, ap=[[NW, P], [1, NW]]),
                single_packet=True,
            )
            nc.sync.dma_start(
                out=QC[:],
                in_=bass.AP(tensor=qc[:].tensor, offset=0, ap=[[32, P], [1, 32]]),
                single_packet=True,
            )
            XF16 = XB[:, 0:XW]
            XLO = XB[:, XW : XW + F]
            XHI = XB[:, XW + F : NW]

            # on-device iota tables (tiny, finish before the DVE chain)
            IL16 = sb.tile([P, NLO], fp16)  # IL16[p,lo] = lo
            IHR = sb.tile([P, NHI, FH], fp16)  # IHR[p,hi,f] = 64*hi
            nc.gpsimd.iota(
                IL16[:], pattern=[[1, NLO]], channel_multiplier=0,
                allow_small_or_imprecise_dtypes=True,
            )
            nc.gpsimd.iota(
                IHR[:], pattern=[[64, NHI], [0, FH]], channel_multiplier=0,
                allow_small_or_imprecise_dtypes=True,
            )

            c1 = nc.const_aps.aps[(fp32, 1.0)]

            # PE warm-up: narrow matmuls keep the HAM clock gate open
            warm = ps.tile([1, 1], fp32)
            for w in range(N_WARM1 + N_WARM2):
                nc.tensor.matmul(
                    warm[:], lhsT=c1[:, 0:1], rhs=c1[:, 0:1], start=True,
                    stop=True, skip_group_check=True,
                )

            Q16 = QC[:, 0:16].rearrange("p (i j) -> p i j", j=K)
            CN16 = QC[:, 16:32]

            EQ = sb.tile([P, F, K, K], fp16)
            CE = sb.tile([P, F, 16], fp16)
            SC = sb.tile([P, F], fp16)
            E = sb.tile([P, F], fp16)
            AEQ = sb.tile([P, NHI, F], fp16)  # transposed: [hi, f]
            Bt = sb.tile([P, F, NLO], fp16)
            A = sb.tile([P, NHI, F], fp16)  # transposed: lhsT slice per f
            acc = ps.tile([NHI, NLO], fp32)

            # EQ[p,f,i,j] = [x[t-j] == q_i]  (t = 16p+f)
            sub = XB[:, K - 1 : XW][:]
            XWIN = bass.AP(
                tensor=sub.tensor,
                offset=sub.offset,
                ap=[sub.ap[0], [1, F], [0, K], [-1, K]],
            )
            with tc.high_priority():
                nc.vector.tensor_tensor(
                    out=EQ[:],
                    in0=XWIN,
                    in1=Q16[:, None, :, :].broadcast_to([P, F, K, K]),
                    op=Alu.is_equal,
                )
                nc.vector.tensor_tensor(
                    out=CE[:],
                    in0=EQ[:].rearrange("p f i j -> p f (i j)"),
                    in1=CN16[:, None, :].broadcast_to([P, F, 16]),
                    op=Alu.mult,
                )
                with nc.allow_low_precision(reason="16 products of |C|<0.1"):
                    for h in range(2):
                        fs = slice(h * FH, (h + 1) * FH)
                        nc.vector.reduce_sum(
                            out=SC[:, fs], in_=CE[:, fs],
                            axis=mybir.AxisListType.X,
                        )
                        nc.scalar.activation(
                            out=E[:, fs], in_=SC[:, fs], func=Act.Exp
                        )

            for h in range(2):
                fs = slice(h * FH, (h + 1) * FH)
                nc.vector.tensor_tensor(
                    out=Bt[:, fs],
                    in0=XLO[:, fs, None].broadcast_to([P, FH, NLO]),
                    in1=IL16[:, None, :].broadcast_to([P, FH, NLO]),
                    op=Alu.is_equal,
                )
                nc.vector.tensor_tensor(
                    out=AEQ[:, :, fs],
                    in0=XHI[:, None, fs].broadcast_to([P, NHI, FH]),
                    in1=IHR[:],
                    op=Alu.is_equal,
                )
                nc.vector.tensor_tensor(
                    out=A[:, :, fs],
                    in0=AEQ[:, :, fs],
                    in1=E[:, None, fs].broadcast_to([P, NHI, FH]),
                    op=Alu.mult,
                )
                for f in range(h * FH, (h + 1) * FH):
                    # t=2047 (p=127, f=15) is excluded from the contraction
                    # entirely -> attn[T-1] = 0 and Z skips it
                    pe = P - 1 if f == F - 1 else P
                    nc.tensor.matmul(
                        acc[:],
                        lhsT=A[0:pe, :, f],
                        rhs=Bt[0:pe, f, :],
                        start=(f == 0),
                        stop=(f == F - 1),
                        skip_group_check=True,
                    )

            OUT = sb.tile([NHI, NLO], fp16)
            nc.vector.tensor_copy(out=OUT[:], in_=acc[:])
            yv = y[:].rearrange("(h l) -> h l", l=NLO)
            nc.sync.dma_start(out=yv[0:16], in_=OUT[0:16, :], single_packet=True)
            nc.scalar.dma_start(
                out=yv[16:32], in_=OUT[16:32, :], single_packet=True
            )
    nc.compile()
    return nc


def _host_prep(x_row: np.ndarray, C: np.ndarray):
    x_row = x_row.astype(np.int32)
    xwin = np.concatenate(
        [np.full(K - 1, -1, np.float16), x_row.astype(np.float16)]
    )
    win = np.lib.stride_tricks.sliding_window_view(xwin, XW)[::F]  # [P, 19]
    lo = (x_row & 63).astype(np.float16).reshape(P, F)
    hi = (x_row - (x_row & 63)).astype(np.float16).reshape(P, F)
    xw = np.concatenate([win, lo, hi], axis=1).reshape(-1)  # [P*51]
    q = x_row[T - 1 : T - 1 - K : -1].astype(np.float16)  # q[i] = x[T-1-i]
    qcrow = np.concatenate([np.repeat(q, K), C.reshape(16).astype(np.float16)])
    return {"xw": xw, "qc": np.tile(qcrow, (P, 1)).reshape(-1)}


_NC_CACHE = {}


def _get_nc():
    if "nc" not in _NC_CACHE:
        _NC_CACHE["nc"] = _build_nc()
    return _NC_CACHE["nc"]


def kernel(x: np.ndarray, C: np.ndarray, _spmd_kwargs: dict | None = None):
    from concourse.bass_utils import run_bass_kernel_spmd

    x = np.asarray(x).astype(np.int32)  # token ids < 2048, exact
    C = np.asarray(C).astype(np.float32)
    assert x.shape == (B, T) and C.shape == (K, K)
    in_maps = [_host_prep(x[b], C) for b in range(B)]
    res = run_bass_kernel_spmd(
        _get_nc(), in_maps, core_ids=list(range(B)), **(_spmd_kwargs or {})
    )
    # y is the unnormalized E-weighted vocab histogram; Z == y.sum()
    hist = np.stack(
        [res.results[b]["y"].astype(np.float32) for b in range(B)], axis=0
    )
    out = hist / hist.sum(axis=1, keepdims=True)
    if _spmd_kwargs:
        kernel.last_results = res
    return out


# revision 53
# speedup vs baseline: 1.0540x; 1.0540x over previous
"""Trainium2 Bass kernel for ConstrainedAttentionModel (sparse_attention).

Full-input contract: kernel(x=[8,2048] int, C=[4,4] f32) -> [8,2048] f32.
Data parallel across 8 NeuronCores: one batch row per core.

Math (per row, T=2048, k=4, V=2048):
  scores[t] = sum_{i,j} C[i,j] * [x[t-j] == x[T-1-i]]   (t-j >= 0)
  scores[T-1] = -1e9; attn = softmax(scores)
  out[v] = sum_t attn[t] * [x[t] == v]

Device strategy (t = 16p + f layout on 128 partitions):
  - all operands arrive pre-converted to fp16 (token ids < 2048 are
    exact in fp16), so the DVE pipeline needs zero prep casts:
      sync DMA:   per-partition [19-token window | lo=x&63 | hi=x-lo]
                  (shifted lag-j window copies are overlapping SBUF
                  views, stride -1 on j)
      scalar DMA: 64B q+C row (single partition)
      gpsimd:     iota rows (lo row + f-replicated hi table)
  - q+C broadcast to 128 partitions via one K=1 PE matmul + one fp16
    PSUM->SBUF copy in an idle DVE slot
  - equality/score chain in fp16 (packed 2x DVE mode), fused reduce,
    single exp -> E fp16
  - vocab one-hot factorized as v = 64*hi + lo:
      A[hi,f] = E[f]*[x>>6 == hi]  transposed layout -> both the
        compare (vs the replicated hi table) and the E-multiply hit the
        2x DVE mode; per-f lhsT slices are strided (LDWEIGHTS is fine)
      B[f,lo] = [x&63 == lo]  contiguous (matmul rhs must stream packed)
    out[hi,lo] = 16 PSUM-accumulated fp16 matmuls at full PE rate
  - the t=T-1 (softmax-masked) position is excluded by contracting only
    127 partitions in the last matmul
  - the kernel returns the UNNORMALIZED histogram; softmax
    normalization happens on host: out = y / y.sum() (Z == sum(y))
"""
import os
import numpy as np
import concourse.bass as bass
import concourse.bacc as bacc
import concourse.tile as tile
from concourse import mybir

P = 128
T = 2048
F = T // P  # 16
K = 4
FH = F // 2  # 8
NHI = 32
NLO = 64
XW = F + K - 1  # 19

fp32 = mybir.dt.float32
fp16 = mybir.dt.float16
Alu = mybir.AluOpType
Act = mybir.ActivationFunctionType

N_WARM1 = int(os.environ.get("KERNEL_N_WARM1", "4"))
N_WARM2 = int(os.environ.get("KERNEL_N_WARM2", "4"))

B = 8


def _build_nc():
    nc = bacc.Bacc()
    NW = XW + 2 * F  # 51: [window 19 | lo 16 | hi 16] per partition
    xw = nc.dram_tensor("xw", [P * NW], fp16, kind="ExternalInput")
    qc = nc.dram_tensor("qc", [P * 32], fp16, kind="ExternalInput")
    y = nc.dram_tensor("y", [T], fp16, kind="ExternalOutput")

    with tile.TileContext(nc) as tc:
        with (
            tc.tile_pool(name="sb", bufs=1) as sb,
            tc.tile_pool(name="ps", bufs=1, space="PSUM") as ps,
        ):
            XB = sb.tile([P, NW], fp16)  # per-partition [win|lo|hi]
            QC = sb.tile([P, 32], fp16)  # host-replicated q+C per partition

            nc.sync.dma_start(
                out=XB[:],
                in_=bass.AP(tensor=xw[:].tensor, offset=0, ap=[[NW, P], [1, NW]]),
                single_packet=True,
            )
            nc.scalar.dma_start(
                out=QC[:],
                in_=bass.AP(tensor=qc[:].tensor, offset=0, ap=[[32, P], [1, 32]]),
                single_packet=True,
            )
            XF16 = XB[:, 0:XW]
            XLO = XB[:, XW : XW + F]
            XHI = XB[:, XW + F : NW]

            # on-device iota tables (tiny, finish before the DVE chain)
            IL16 = sb.tile([P, NLO], fp16)  # IL16[p,lo] = lo
            IHR = sb.tile([P, NHI, FH], fp16)  # IHR[p,hi,f] = 64*hi
            nc.gpsimd.iota(
                IL16[:], pattern=[[1, NLO]], channel_multiplier=0,
                allow_small_or_imprecise_dtypes=True,
            )
            nc.gpsimd.iota(
                IHR[:], pattern=[[64, NHI], [0, FH]], channel_multiplier=0,
                allow_small_or_imprecise_dtypes=True,
            )

            c1 = nc.const_aps.aps[(fp32, 1.0)]

            # PE warm-up: narrow matmuls keep the HAM clock gate open
            warm = ps.tile([1, 1], fp32)
            for w in range(N_WARM1 + N_WARM2):
                nc.tensor.matmul(
                    warm[:], lhsT=c1[:, 0:1], rhs=c1[:, 0:1], start=True,
                    stop=True, skip_group_check=True,
                )

            Q16 = QC[:, 0:16].rearrange("p (i j) -> p i j", j=K)
            CN16 = QC[:, 16:32]

            EQ = sb.tile([P, F, K, K], fp16)
            CE = sb.tile([P, F, 16], fp16)
            SC = sb.tile([P, F], fp16)
            E = sb.tile([P, F], fp16)
            AEQ = sb.tile([P, NHI, F], fp16)  # transposed: [hi, f]
            Bt = sb.tile([P, F, NLO], fp16)
            A = sb.tile([P, NHI, F], fp16)  # transposed: lhsT slice per f
            acc = ps.tile([NHI, NLO], fp32)

            # EQ[p,f,i,j] = [x[t-j] == q_i]  (t = 16p+f)
            sub = XB[:, K - 1 : XW][:]
            XWIN = bass.AP(
                tensor=sub.tensor,
                offset=sub.offset,
                ap=[sub.ap[0], [1, F], [0, K], [-1, K]],
            )
            with tc.high_priority():
                nc.vector.tensor_tensor(
                    out=EQ[:],
                    in0=XWIN,
                    in1=Q16[:, None, :, :].broadcast_to([P, F, K, K]),
                    op=Alu.is_equal,
                )
                nc.vector.tensor_tensor(
                    out=CE[:],
                    in0=EQ[:].rearrange("p f i j -> p f (i j)"),
                    in1=CN16[:, None, :].broadcast_to([P, F, 16]),
                    op=Alu.mult,
                )
                with nc.allow_low_precision(reason="16 products of |C|<0.1"):
                    for h in range(2):
                        fs = slice(h * FH, (h + 1) * FH)
                        nc.vector.reduce_sum(
                            out=SC[:, fs], in_=CE[:, fs],
                            axis=mybir.AxisListType.X,
                        )
                        nc.scalar.activation(
                            out=E[:, fs], in_=SC[:, fs], func=Act.Exp
                        )

            for h in range(2):
                fs = slice(h * FH, (h + 1) * FH)
                nc.vector.tensor_tensor(
                    out=Bt[:, fs],
                    in0=XLO[:, fs, None].broadcast_to([P, FH, NLO]),
                    in1=IL16[:, None, :].broadcast_to([P, FH, NLO]),
                    op=Alu.is_equal,
                )
                nc.vector.tensor_tensor(
                    out=AEQ[:, :, fs],
                    in0=XHI[:, None, fs].broadcast_to([P, NHI, FH]),
                    in1=IHR[:],
                    op=Alu.is_equal,
                )
                nc.vector.tensor_tensor(
                    out=A[:, :, fs],
                    in0=AEQ[:, :, fs],
                    in1=E[:, None, fs].broadcast_to([P, NHI, FH]),
                    op=Alu.mult,
                )
                for f in range(h * FH, (h + 1) * FH):
                    # t=2047 (p=127, f=15) is excluded from the contraction
                    # entirely -> attn[T-1] = 0 and Z skips it
                    pe = P - 1 if f == F - 1 else P
                    nc.tensor.matmul(
                        acc[:],
                        lhsT=A[0:pe, :, f],
                        rhs=Bt[0:pe, f, :],
                        start=(f == 0),
                        stop=(f == F - 1),
                        skip_group_check=True,
                    )

            OUT = sb.tile([NHI, NLO], fp16)
            nc.vector.tensor_copy(out=OUT[:], in_=acc[:])
            yv = y[:].rearrange("(h l) -> h l", l=NLO)
            nc.sync.dma_start(out=yv[0:16], in_=OUT[0:16, :], single_packet=True)
            nc.scalar.dma_start(
                out=yv[16:32], in_=OUT[16:32, :], single_packet=True
            )
    nc.compile()
    return nc


def _host_prep(x_row: np.ndarray, C: np.ndarray):
    x_row = x_row.astype(np.int32)
    xwin = np.concatenate(
        [np.full(K - 1, -1, np.float16), x_row.astype(np.float16)]
    )
    win = np.lib.stride_tricks.sliding_window_view(xwin, XW)[::F]  # [P, 19]
    lo = (x_row & 63).astype(np.float16).reshape(P, F)
    hi = (x_row - (x_row & 63)).astype(np.float16).reshape(P, F)
    xw = np.concatenate([win, lo, hi], axis=1).reshape(-1)  # [P*51]
    q = x_row[T - 1 : T - 1 - K : -1].astype(np.float16)  # q[i] = x[T-1-i]
    qcrow = np.concatenate([np.repeat(q, K), C.reshape(16).astype(np.float16)])
    return {"xw": xw, "qc": np.tile(qcrow, (P, 1)).reshape(-1)}


_NC_CACHE = {}


def _get_nc():
    if "nc" not in _NC_CACHE:
        _NC_CACHE["nc"] = _build_nc()
    return _NC_CACHE["nc"]


def kernel(x: np.ndarray, C: np.ndarray, _spmd_kwargs: dict | None = None):
    from concourse.bass_utils import run_bass_kernel_spmd

    x = np.asarray(x).astype(np.int32)  # token ids < 2048, exact
    C = np.asarray(C).astype(np.float32)
    assert x.shape == (B, T) and C.shape == (K, K)
    in_maps = [_host_prep(x[b], C) for b in range(B)]
    res = run_bass_kernel_spmd(
        _get_nc(), in_maps, core_ids=list(range(B)), **(_spmd_kwargs or {})
    )
    # y is the unnormalized E-weighted vocab histogram; Z == y.sum()
    hist = np.stack(
        [res.results[b]["y"].astype(np.float32) for b in range(B)], axis=0
    )
    out = hist / hist.sum(axis=1, keepdims=True)
    if _spmd_kwargs:
        kernel.last_results = res
    return out


# revision 54
# speedup vs baseline: 1.1182x; 1.0609x over previous
"""Trainium2 Bass kernel for ConstrainedAttentionModel (sparse_attention).

Full-input contract: kernel(x=[8,2048] int, C=[4,4] f32) -> [8,2048] f32.
Data parallel across 8 NeuronCores: one batch row per core.

Math (per row, T=2048, k=4, V=2048):
  scores[t] = sum_{i,j} C[i,j] * [x[t-j] == x[T-1-i]]   (t-j >= 0)
  scores[T-1] = -1e9; attn = softmax(scores)
  out[v] = sum_t attn[t] * [x[t] == v]

Device strategy (t = 16p + f layout on 128 partitions):
  - all operands arrive pre-converted to fp16 (token ids < 2048 are
    exact in fp16), so the DVE pipeline needs zero prep casts:
      sync DMA:   per-partition [19-token window | lo=x&63 | hi=x-lo]
                  (shifted lag-j window copies are overlapping SBUF
                  views, stride -1 on j)
      scalar DMA: q+C replicated per partition by the host (64B/row --
                  cheaper and ~0.6us earlier than a PE broadcast)
      gpsimd:     iota rows (lo row + f-replicated hi table)
  - equality/score chain in fp16 (packed 2x DVE mode), fused reduce,
    single exp -> E fp16
  - vocab one-hot factorized as v = 64*hi + lo:
      A[hi,f] = E[f]*[x>>6 == hi]  transposed layout -> both the
        compare (vs the replicated hi table) and the E-multiply hit the
        2x DVE mode; per-f lhsT slices are strided (LDWEIGHTS is fine)
      B[f,lo] = [x&63 == lo]  contiguous (matmul rhs must stream packed)
    out[hi,lo] = 16 PSUM-accumulated fp16 matmuls at full PE rate
  - the t=T-1 (softmax-masked) position is excluded by contracting only
    127 partitions in the last matmul
  - the kernel returns the UNNORMALIZED histogram; softmax
    normalization happens on host: out = y / y.sum() (Z == sum(y))
"""
import os
import numpy as np
import concourse.bass as bass
import concourse.bacc as bacc
import concourse.tile as tile
from concourse import mybir

P = 128
T = 2048
F = T // P  # 16
K = 4
FH = F // 2  # 8
NHI = 32
NLO = 64
XW = F + K - 1  # 19

fp32 = mybir.dt.float32
fp16 = mybir.dt.float16
Alu = mybir.AluOpType
Act = mybir.ActivationFunctionType

N_WARM1 = int(os.environ.get("KERNEL_N_WARM1", "4"))
N_WARM2 = int(os.environ.get("KERNEL_N_WARM2", "4"))

B = 8


def _build_nc():
    nc = bacc.Bacc()
    NW = XW + 2 * F  # 51: [window 19 | lo 16 | hi 16] per partition
    xw = nc.dram_tensor("xw", [P * NW], fp16, kind="ExternalInput")
    qc = nc.dram_tensor("qc", [P * 32], fp16, kind="ExternalInput")
    y = nc.dram_tensor("y", [T], fp16, kind="ExternalOutput")

    with tile.TileContext(nc) as tc:
        with (
            tc.tile_pool(name="sb", bufs=1) as sb,
            tc.tile_pool(name="ps", bufs=1, space="PSUM") as ps,
        ):
            XB = sb.tile([P, NW], fp16)  # per-partition [win|lo|hi]
            QC = sb.tile([P, 32], fp16)  # host-replicated q+C per partition

            nc.sync.dma_start(
                out=XB[:],
                in_=bass.AP(tensor=xw[:].tensor, offset=0, ap=[[NW, P], [1, NW]]),
                single_packet=True,
            )
            nc.scalar.dma_start(
                out=QC[:],
                in_=bass.AP(tensor=qc[:].tensor, offset=0, ap=[[32, P], [1, 32]]),
                single_packet=True,
            )
            XF16 = XB[:, 0:XW]
            XLO = XB[:, XW : XW + F]
            XHI = XB[:, XW + F : NW]

            # on-device iota tables (tiny, finish before the DVE chain)
            IL16 = sb.tile([P, NLO], fp16)  # IL16[p,lo] = lo
            IHR = sb.tile([P, NHI, FH], fp16)  # IHR[p,hi,f] = 64*hi
            nc.gpsimd.iota(
                IL16[:], pattern=[[1, NLO]], channel_multiplier=0,
                allow_small_or_imprecise_dtypes=True,
            )
            nc.gpsimd.iota(
                IHR[:], pattern=[[64, NHI], [0, FH]], channel_multiplier=0,
                allow_small_or_imprecise_dtypes=True,
            )

            c1 = nc.const_aps.aps[(fp32, 1.0)]

            # PE warm-up: narrow matmuls keep the HAM clock gate open
            warm = ps.tile([1, 1], fp32)
            for w in range(N_WARM1 + N_WARM2):
                nc.tensor.matmul(
                    warm[:], lhsT=c1[:, 0:1], rhs=c1[:, 0:1], start=True,
                    stop=True, skip_group_check=True,
                )

            Q16 = QC[:, 0:16].rearrange("p (i j) -> p i j", j=K)
            CN16 = QC[:, 16:32]

            EQ = sb.tile([P, F, K, K], fp16)
            CE = sb.tile([P, F, 16], fp16)
            SC = sb.tile([P, F], fp16)
            E = sb.tile([P, F], fp16)
            AEQ = sb.tile([P, NHI, F], fp16)  # transposed: [hi, f]
            Bt = sb.tile([P, F, NLO], fp16)
            A = sb.tile([P, NHI, F], fp16)  # transposed: lhsT slice per f
            acc = ps.tile([NHI, NLO], fp32)

            # EQ[p,f,i,j] = [x[t-j] == q_i]  (t = 16p+f)
            sub = XB[:, K - 1 : XW][:]
            XWIN = bass.AP(
                tensor=sub.tensor,
                offset=sub.offset,
                ap=[sub.ap[0], [1, F], [0, K], [-1, K]],
            )
            with tc.high_priority():
                nc.vector.tensor_tensor(
                    out=EQ[:],
                    in0=XWIN,
                    in1=Q16[:, None, :, :].broadcast_to([P, F, K, K]),
                    op=Alu.is_equal,
                )
                nc.vector.tensor_tensor(
                    out=CE[:],
                    in0=EQ[:].rearrange("p f i j -> p f (i j)"),
                    in1=CN16[:, None, :].broadcast_to([P, F, 16]),
                    op=Alu.mult,
                )
                with nc.allow_low_precision(reason="16 products of |C|<0.1"):
                    for h in range(2):
                        fs = slice(h * FH, (h + 1) * FH)
                        nc.vector.reduce_sum(
                            out=SC[:, fs], in_=CE[:, fs],
                            axis=mybir.AxisListType.X,
                        )
                        nc.scalar.activation(
                            out=E[:, fs], in_=SC[:, fs], func=Act.Exp
                        )

            for h in range(2):
                fs = slice(h * FH, (h + 1) * FH)
                nc.vector.tensor_tensor(
                    out=Bt[:, fs],
                    in0=XLO[:, fs, None].broadcast_to([P, FH, NLO]),
                    in1=IL16[:, None, :].broadcast_to([P, FH, NLO]),
                    op=Alu.is_equal,
                )
                nc.vector.tensor_tensor(
                    out=AEQ[:, :, fs],
                    in0=XHI[:, None, fs].broadcast_to([P, NHI, FH]),
                    in1=IHR[:],
                    op=Alu.is_equal,
                )
                nc.vector.tensor_tensor(
                    out=A[:, :, fs],
                    in0=AEQ[:, :, fs],
                    in1=E[:, None, fs].broadcast_to([P, NHI, FH]),
                    op=Alu.mult,
                )
                for f in range(h * FH, (h + 1) * FH):
                    # t=2047 (p=127, f=15) is excluded from the contraction
                    # entirely -> attn[T-1] = 0 and Z skips it
                    pe = P - 1 if f == F - 1 else P
                    nc.tensor.matmul(
                        acc[:],
                        lhsT=A[0:pe, :, f],
                        rhs=Bt[0:pe, f, :],
                        start=(f == 0),
                        stop=(f == F - 1),
                        skip_group_check=True,
                    )

            OUT = sb.tile([NHI, NLO], fp16)
            nc.vector.tensor_copy(out=OUT[:], in_=acc[:])
            yv = y[:].rearrange("(h l) -> h l", l=NLO)
            nc.sync.dma_start(out=yv[0:16], in_=OUT[0:16, :], single_packet=True)
            nc.scalar.dma_start(
                out=yv[16:32], in_=OUT[16:32, :], single_packet=True
            )
    nc.compile()
    return nc


def _host_prep(x_row: np.ndarray, C: np.ndarray):
    x_row = x_row.astype(np.int32)
    xwin = np.concatenate(
        [np.full(K - 1, -1, np.float16), x_row.astype(np.float16)]
    )
    win = np.lib.stride_tricks.sliding_window_view(xwin, XW)[::F]  # [P, 19]
    lo = (x_row & 63).astype(np.float16).reshape(P, F)
    hi = (x_row - (x_row & 63)).astype(np.float16).reshape(P, F)
    xw = np.concatenate([win, lo, hi], axis=1).reshape(-1)  # [P*51]
    q = x_row[T - 1 : T - 1 - K : -1].astype(np.float16)  # q[i] = x[T-1-i]
    qcrow = np.concatenate([np.repeat(q, K), C.reshape(16).astype(np.float16)])
    return {"xw": xw, "qc": np.tile(qcrow, (P, 1)).reshape(-1)}


_NC_CACHE = {}


def _get_nc():
    if "nc" not in _NC_CACHE:
        _NC_CACHE["nc"] = _build_nc()
    return _NC_CACHE["nc"]


def kernel(x: np.ndarray, C: np.ndarray, _spmd_kwargs: dict | None = None):
    from concourse.bass_utils import run_bass_kernel_spmd

    x = np.asarray(x).astype(np.int32)  # token ids < 2048, exact
    C = np.asarray(C).astype(np.float32)
    assert x.shape == (B, T) and C.shape == (K, K)
    in_maps = [_host_prep(x[b], C) for b in range(B)]
    res = run_bass_kernel_spmd(
        _get_nc(), in_maps, core_ids=list(range(B)), **(_spmd_kwargs or {})
    )
    # y is the unnormalized E-weighted vocab histogram; Z == y.sum()
    hist = np.stack(
        [res.results[b]["y"].astype(np.float32) for b in range(B)], axis=0
    )
    out = hist / hist.sum(axis=1, keepdims=True)
    if _spmd_kwargs:
        kernel.last_results = res
    return out
